# revision 21
# baseline (speedup 1.0000x reference)
"""AWQ 4-bit quantized linear layer on 8 Trainium2 NeuronCores.

Problem: out = x @ dequant(qweight, scales, qzeros) + bias
  x       [8192, 4096] fp16   (replicated to all cores, pre-transposed on host)
  qweight [4096, 1536] int32  (8x int4 nibbles packed along out_features)
  scales  [32, 12288]  fp16   (group_size=128 along in_features)
  qzeros  [32, 1536]   int32  (packed like qweight)
  bias    [12288]      fp16
  out     [8192, 12288] fp16

Sharding: tensor-parallel colwise. out_features 12288 -> 8 shards of 1536
(192 packed int32 columns). Each core computes out[:, shard] independently;
host concatenates. No collectives. x is replicated, transposed on host so
the contraction dim lands on SBUF partitions with plain (non-xbar) DMAs.

Per-core kernel:
  1. Load all packed qweight in one DMA ([128, KT, C] int32, 3 MiB).
  2. Unpack qzeros on G partitions, compute zs = z * s, stage [s | zs]
     rows to a DRAM scratch; per k-tile one 0-stride-partition DMA
     broadcasts the group's [s | zs] row to 128 partitions.
  3. Dequantize the full weight shard once into resident SBUF
     (32 tiles [128, 1536] fp16 = 96 KiB/partition):
     8x nibble isolation ops per k-tile, then w = wq * s_b - zs_b.
  4. Stream xT tiles [128, MS]; per m-tile/o-tile accumulate 32 matmuls
     in PSUM; evict with fused bias add (f32 psum + f16 bias -> f16); DMA out.
"""

import sys

for p in ("/opt/trn_rl_repo", "/opt/pypackages"):
    if p not in sys.path:
        sys.path.insert(0, p)

import numpy as np

import concourse.bacc as bacc
import concourse.bass as bass
import concourse.mybir as mybir
from concourse.tile import TileContext

f16 = mybir.dt.float16
f32 = mybir.dt.float32
i32 = mybir.dt.int32
Alu = mybir.AluOpType

N_CORES = 8
M_FULL, K_FULL, O_FULL = 8192, 4096, 12288
GROUP_SIZE = 128
PACK = 8  # int4 values per int32

O_SHARD = O_FULL // N_CORES        # 1536
C_SHARD = O_SHARD // PACK          # 192


def build_nc(M=M_FULL, K=K_FULL, O=O_SHARD, MS=512, xt_bufs=52,
             unpack_mode="staged", qw_chunk=8):
    """Build the per-core Bass program (SPMD: same program on all cores).

    Unpack is staged: (q >> 4j) & 0xF into int32 staging (bitvec ALU ops
    cannot cast on write), then one arithmetic op casts int32 -> f16.
    """
    KT = K // 128                  # k-tiles == quant groups per shard
    G = K // GROUP_SIZE
    assert KT == G, "kernel assumes group_size == 128 == k-tile"
    C = O // PACK
    OT = O // 512                  # o-tiles of 512
    NMS = M // MS                  # number of m-superchunks
    MT = MS // 128                 # m-tiles per superchunk

    # Bacc (not Bass): its compile() pipeline legalizes per-instruction
    # semaphore waits (generate_event_semaphores / move_matmul_waits_to_
    # ldweights) so walrus' per-struct sync-wait limits are respected.
    nc = bacc.Bacc("TRN2")
    xt_in = nc.dram_tensor("xt", [K, M], f16, kind="ExternalInput")
    qw = nc.dram_tensor("qw", [K, C], i32, kind="ExternalInput")
    scales = nc.dram_tensor("scales", [G, O], f16, kind="ExternalInput")
    qzeros = nc.dram_tensor("qzeros", [G, C], i32, kind="ExternalInput")
    bias = nc.dram_tensor("bias", [1, O], f16, kind="ExternalInput")
    out = nc.dram_tensor("out", [M, O], f16, kind="ExternalOutput")

    with TileContext(nc) as tc:
        with (
            tc.tile_pool(name="wres", bufs=KT) as w_pool,
            tc.tile_pool(name="xt", bufs=xt_bufs) as xt_pool,
            tc.tile_pool(name="qall", bufs=1) as qall_pool,
            tc.tile_pool(name="bc", bufs=3) as bc_pool,
            tc.tile_pool(name="meta", bufs=1) as meta_pool,
            tc.tile_pool(name="obuf", bufs=2) as o_pool,
            tc.tile_pool(name="scratch", bufs=1, space="DRAM") as dram_pool,
            tc.tile_pool(name="psum", bufs=8, space="PSUM") as psum_pool,
        ):
            assert unpack_mode == "staged"

            # ---- kick off the first weight chunk + x tiles immediately ----
            qw_r = qw.rearrange("(t p) c -> p t c", p=128)
            qw_c0 = qall_pool.tile([128, qw_chunk, C], i32, tag="qwc", bufs=2)
            nc.sync.dma_start(qw_c0[:], qw_r[:, 0:qw_chunk, :])
            xts0 = []
            for t in range(KT):
                xt = xt_pool.tile([128, MS], f16, tag="xt", name="xt")
                nc.sync.dma_start(xt[:], xt_in[t * 128:(t + 1) * 128, 0:MS])
                xts0.append(xt)

            # ---- group metadata on G partitions ----
            # ssz row layout: [:, :O] = s, [:, O:] = zs = z * s
            qz_sb = meta_pool.tile([G, C], i32, tag="qz")
            nc.sync.dma_start(qz_sb[:], qzeros[:, :])
            ssz_sb = meta_pool.tile([G, 2 * O], f16, tag="ssz")
            nc.sync.dma_start(ssz_sb[:, :O], scales[:, :])
            zq_i = meta_pool.tile([G, O], i32, tag="zqi")
            zv = zq_i.rearrange("p (c j) -> p c j", j=PACK)
            for j in range(PACK):
                nc.vector.tensor_scalar(
                    zv[:, :, j], qz_sb[:], 4 * j, 0xF,
                    Alu.logical_shift_right, Alu.bitwise_and,
                )
            # cast int32 zeros -> f16 into the zs half, then scale in place
            nc.vector.tensor_scalar(
                ssz_sb[:, O:], zq_i[:], 0, None, Alu.add)
            nc.vector.tensor_tensor(
                ssz_sb[:, O:], ssz_sb[:, O:], ssz_sb[:, :O], Alu.mult)
            ssz_dram = dram_pool.tile([G, 2 * O], f16, tag="sszd")
            nc.sync.dma_start(ssz_dram[:, :], ssz_sb[:])

            # ---- bias broadcast [128, O] ----
            bias_b = meta_pool.tile([128, O], f16, tag="biasb")
            nc.sync.dma_start(bias_b[:], bias[0, :].partition_broadcast(128))

            # ---- dequantize w shard into resident SBUF tiles ----
            # packed weights arrive in chunks of qw_chunk k-tiles per DMA
            w_tiles = []
            qw_c = qw_c0
            for t in range(KT):
                if t % qw_chunk == 0 and t > 0:
                    qw_c = qall_pool.tile([128, qw_chunk, C], i32,
                                          tag="qwc", bufs=2)
                    nc.sync.dma_start(qw_c[:], qw_r[:, t:t + qw_chunk, :])
                ssz_b = bc_pool.tile([128, 2 * O], f16, tag="sszb", bufs=2)
                nc.sync.dma_start(
                    ssz_b[:], ssz_dram[t, :].partition_broadcast(128))
                w_t = w_pool.tile([128, O], f16, tag="w")
                wq_i = bc_pool.tile([128, O], i32, tag="wqi", bufs=2)
                wqv = wq_i.rearrange("p (c j) -> p c j", j=PACK)
                for j in range(PACK):
                    nc.vector.tensor_scalar(
                        wqv[:, :, j], qw_c[:, t % qw_chunk, :], 4 * j, 0xF,
                        Alu.logical_shift_right, Alu.bitwise_and,
                    )
                # w = wq * s (mixed int32 x f16 TT casts on write; DVE only —
                # Pool integer TT requires same-size input dtypes), then
                # w -= z*s on Pool to parallelize the dequant ramp
                nc.vector.tensor_tensor(
                    w_t[:], wq_i[:], ssz_b[:, :O], Alu.mult)
                nc.gpsimd.tensor_tensor(
                    w_t[:], w_t[:], ssz_b[:, O:], Alu.subtract)
                w_tiles.append(w_t)

            # ---- main loop: stream xT, accumulate matmuls, evict ----
            for ms in range(NMS):
                if ms == 0:
                    xts = xts0
                else:
                    xts = []
                    for t in range(KT):
                        xt = xt_pool.tile([128, MS], f16, tag="xt", name="xt")
                        nc.sync.dma_start(
                            xt[:],
                            xt_in[t * 128:(t + 1) * 128,
                                  ms * MS:(ms + 1) * MS],
                        )
                        xts.append(xt)
                for mi in range(MT):
                    out_sb = o_pool.tile([128, O], f16, tag="osb")
                    for o in range(OT):
                        ps = psum_pool.tile([128, 512], f32, tag="ps")
                        for t in range(KT):
                            nc.tensor.matmul(
                                ps[:],
                                xts[t][:, mi * 128:(mi + 1) * 128],
                                w_tiles[t][:, o * 512:(o + 1) * 512],
                                start=(t == 0),
                                stop=(t == KT - 1),
                            )
                        # evict on ACT (frees the PSUM bank + DVE), then
                        # add bias in place on DVE (f16 SBUF 2x mode)
                        nc.scalar.copy(
                            out_sb[:, o * 512:(o + 1) * 512], ps[:])
                        nc.vector.tensor_tensor(
                            out_sb[:, o * 512:(o + 1) * 512],
                            out_sb[:, o * 512:(o + 1) * 512],
                            bias_b[:, o * 512:(o + 1) * 512], Alu.add,
                        )
                    m0 = ms * MS + mi * 128
                    nc.sync.dma_start(out[m0:m0 + 128, :], out_sb[:])

    if not nc.is_finalized():
        nc.finalize()
    return nc


def _shard_inputs(x, qweight, scales, qzeros, bias):
    xt_full = np.ascontiguousarray(np.asarray(x).T)  # [K, M], replicated
    in_maps = []
    for c in range(N_CORES):
        so = slice(c * O_SHARD, (c + 1) * O_SHARD)
        sc = slice(c * C_SHARD, (c + 1) * C_SHARD)
        in_maps.append({
            "xt": xt_full,
            "qw": np.ascontiguousarray(qweight[:, sc]),
            "scales": np.ascontiguousarray(scales[:, so]),
            "qzeros": np.ascontiguousarray(qzeros[:, sc]),
            "bias": np.ascontiguousarray(bias[so]).reshape(1, -1),
        })
    return in_maps


_CACHED_NC = None


def kernel(x, qweight, scales, qzeros, bias):
    from concourse.bass_utils import run_bass_kernel_spmd

    global _CACHED_NC
    if _CACHED_NC is None:
        _CACHED_NC = build_nc()
    nc = _CACHED_NC

    in_maps = _shard_inputs(x, qweight, scales, qzeros, bias)
    res = run_bass_kernel_spmd(nc, in_maps, core_ids=list(range(N_CORES)))
    return np.concatenate([r["out"] for r in res.results], axis=1)


# revision 26
# speedup vs baseline: 1.0030x; 1.0030x over previous
"""AWQ 4-bit quantized linear layer on 8 Trainium2 NeuronCores.

Problem: out = x @ dequant(qweight, scales, qzeros) + bias
  x       [8192, 4096] fp16   (replicated to all cores, pre-transposed on host)
  qweight [4096, 1536] int32  (8x int4 nibbles packed along out_features)
  scales  [32, 12288]  fp16   (group_size=128 along in_features)
  qzeros  [32, 1536]   int32  (packed like qweight)
  bias    [12288]      fp16
  out     [8192, 12288] fp16

Sharding: tensor-parallel colwise. out_features 12288 -> 8 shards of 1536
(192 packed int32 columns). Each core computes out[:, shard] independently;
host concatenates. No collectives. x is replicated, transposed on host so
the contraction dim lands on SBUF partitions with plain (non-xbar) DMAs.

Per-core kernel:
  1. Load all packed qweight in one DMA ([128, KT, C] int32, 3 MiB).
  2. Unpack qzeros on G partitions, compute zs = z * s, stage [s | zs]
     rows to a DRAM scratch; per k-tile one 0-stride-partition DMA
     broadcasts the group's [s | zs] row to 128 partitions.
  3. Dequantize the full weight shard once into resident SBUF
     (32 tiles [128, 1536] fp16 = 96 KiB/partition):
     8x nibble isolation ops per k-tile, then w = wq * s_b - zs_b.
  4. Stream xT tiles [128, MS]; per m-tile/o-tile accumulate 32 matmuls
     in PSUM; evict with fused bias add (f32 psum + f16 bias -> f16); DMA out.
"""

import sys

for p in ("/opt/trn_rl_repo", "/opt/pypackages"):
    if p not in sys.path:
        sys.path.insert(0, p)

import numpy as np

import concourse.bacc as bacc
import concourse.bass as bass
import concourse.mybir as mybir
from concourse.tile import TileContext

f16 = mybir.dt.float16
f32 = mybir.dt.float32
i32 = mybir.dt.int32
Alu = mybir.AluOpType

N_CORES = 8
M_FULL, K_FULL, O_FULL = 8192, 4096, 12288
GROUP_SIZE = 128
PACK = 8  # int4 values per int32

O_SHARD = O_FULL // N_CORES        # 1536
C_SHARD = O_SHARD // PACK          # 192


def build_nc(M=M_FULL, K=K_FULL, O=O_SHARD, MS=512, xt_bufs=52,
             unpack_mode="staged", qw_chunk=8):
    """Build the per-core Bass program (SPMD: same program on all cores).

    Unpack is staged: (q >> 4j) & 0xF into int32 staging (bitvec ALU ops
    cannot cast on write), then one arithmetic op casts int32 -> f16.
    """
    KT = K // 128                  # k-tiles == quant groups per shard
    G = K // GROUP_SIZE
    assert KT == G, "kernel assumes group_size == 128 == k-tile"
    C = O // PACK
    OT = O // 512                  # o-tiles of 512
    NMS = M // MS                  # number of m-superchunks
    MT = MS // 128                 # m-tiles per superchunk

    # Bacc (not Bass): its compile() pipeline legalizes per-instruction
    # semaphore waits (generate_event_semaphores / move_matmul_waits_to_
    # ldweights) so walrus' per-struct sync-wait limits are respected.
    nc = bacc.Bacc("TRN2")
    xt_in = nc.dram_tensor("xt", [K, M], f16, kind="ExternalInput")
    qw = nc.dram_tensor("qw", [K, C], i32, kind="ExternalInput")
    scales = nc.dram_tensor("scales", [G, O], f16, kind="ExternalInput")
    qzeros = nc.dram_tensor("qzeros", [G, C], i32, kind="ExternalInput")
    bias = nc.dram_tensor("bias", [1, O], f16, kind="ExternalInput")
    out = nc.dram_tensor("out", [M, O], f16, kind="ExternalOutput")

    with TileContext(nc) as tc:
        with (
            tc.tile_pool(name="wres", bufs=KT) as w_pool,
            tc.tile_pool(name="xt", bufs=xt_bufs) as xt_pool,
            tc.tile_pool(name="qall", bufs=1) as qall_pool,
            tc.tile_pool(name="bc", bufs=3) as bc_pool,
            tc.tile_pool(name="meta", bufs=1) as meta_pool,
            tc.tile_pool(name="obuf", bufs=2) as o_pool,
            tc.tile_pool(name="scratch", bufs=1, space="DRAM") as dram_pool,
            tc.tile_pool(name="psum", bufs=8, space="PSUM") as psum_pool,
        ):
            assert unpack_mode == "staged"

            # dequant-phase DMAs ride the ACT HWDGE ring (nc.scalar) so they
            # never queue behind the bulk xt stream on the SP ring
            qw_r = qw.rearrange("(t p) c -> p t c", p=128)
            qw_c0 = qall_pool.tile([128, qw_chunk, C], i32, tag="qwc", bufs=2)
            nc.scalar.dma_start(qw_c0[:], qw_r[:, 0:qw_chunk, :])

            # ---- group metadata on G partitions ----
            # ssz row layout: [:, :O] = s, [:, O:] = zs = z * s
            qz_sb = meta_pool.tile([G, C], i32, tag="qz")
            nc.scalar.dma_start(qz_sb[:], qzeros[:, :])
            ssz_sb = meta_pool.tile([G, 2 * O], f16, tag="ssz")
            nc.scalar.dma_start(ssz_sb[:, :O], scales[:, :])
            zq_i = meta_pool.tile([G, O], i32, tag="zqi")
            zv = zq_i.rearrange("p (c j) -> p c j", j=PACK)
            for j in range(PACK):
                nc.vector.tensor_scalar(
                    zv[:, :, j], qz_sb[:], 4 * j, 0xF,
                    Alu.logical_shift_right, Alu.bitwise_and,
                )
            # cast int32 zeros -> f16 into the zs half, then scale in place
            nc.vector.tensor_scalar(
                ssz_sb[:, O:], zq_i[:], 0, None, Alu.add)
            nc.vector.tensor_tensor(
                ssz_sb[:, O:], ssz_sb[:, O:], ssz_sb[:, :O], Alu.mult)
            ssz_dram = dram_pool.tile([G, 2 * O], f16, tag="sszd")
            nc.scalar.dma_start(ssz_dram[:, :], ssz_sb[:])

            # superchunk-0 x tiles can start now on the SP ring
            xts0 = []
            for t in range(KT):
                xt = xt_pool.tile([128, MS], f16, tag="xt", name="xt")
                nc.sync.dma_start(xt[:], xt_in[t * 128:(t + 1) * 128, 0:MS])
                xts0.append(xt)

            # ---- bias broadcast [128, O] ----
            bias_b = meta_pool.tile([128, O], f16, tag="biasb")
            nc.scalar.dma_start(bias_b[:], bias[0, :].partition_broadcast(128))

            # ---- dequantize w shard into resident SBUF tiles ----
            # packed weights arrive in chunks of qw_chunk k-tiles per DMA
            w_tiles = []
            qw_c = qw_c0
            for t in range(KT):
                if t % qw_chunk == 0 and t > 0:
                    qw_c = qall_pool.tile([128, qw_chunk, C], i32,
                                          tag="qwc", bufs=2)
                    nc.scalar.dma_start(qw_c[:], qw_r[:, t:t + qw_chunk, :])
                ssz_b = bc_pool.tile([128, 2 * O], f16, tag="sszb", bufs=2)
                nc.scalar.dma_start(
                    ssz_b[:], ssz_dram[t, :].partition_broadcast(128))
                w_t = w_pool.tile([128, O], f16, tag="w")
                wq_i = bc_pool.tile([128, O], i32, tag="wqi", bufs=2)
                wqv = wq_i.rearrange("p (c j) -> p c j", j=PACK)
                for j in range(PACK):
                    nc.vector.tensor_scalar(
                        wqv[:, :, j], qw_c[:, t % qw_chunk, :], 4 * j, 0xF,
                        Alu.logical_shift_right, Alu.bitwise_and,
                    )
                # w = wq * s (mixed int32 x f16 TT casts on write; DVE only —
                # Pool integer TT requires same-size input dtypes), then
                # w -= z*s on Pool to parallelize the dequant ramp
                nc.vector.tensor_tensor(
                    w_t[:], wq_i[:], ssz_b[:, :O], Alu.mult)
                nc.gpsimd.tensor_tensor(
                    w_t[:], w_t[:], ssz_b[:, O:], Alu.subtract)
                w_tiles.append(w_t)

            # ---- main loop: stream xT, accumulate matmuls, evict ----
            for ms in range(NMS):
                if ms == 0:
                    xts = xts0
                else:
                    xts = []
                    for t in range(KT):
                        xt = xt_pool.tile([128, MS], f16, tag="xt", name="xt")
                        nc.sync.dma_start(
                            xt[:],
                            xt_in[t * 128:(t + 1) * 128,
                                  ms * MS:(ms + 1) * MS],
                        )
                        xts.append(xt)
                for mi in range(MT):
                    out_sb = o_pool.tile([128, O], f16, tag="osb")
                    for o in range(OT):
                        ps = psum_pool.tile([128, 512], f32, tag="ps")
                        for t in range(KT):
                            nc.tensor.matmul(
                                ps[:],
                                xts[t][:, mi * 128:(mi + 1) * 128],
                                w_tiles[t][:, o * 512:(o + 1) * 512],
                                start=(t == 0),
                                stop=(t == KT - 1),
                            )
                        # evict on ACT (frees the PSUM bank + DVE), then
                        # add bias in place on DVE (f16 SBUF 2x mode)
                        nc.scalar.copy(
                            out_sb[:, o * 512:(o + 1) * 512], ps[:])
                        nc.vector.tensor_tensor(
                            out_sb[:, o * 512:(o + 1) * 512],
                            out_sb[:, o * 512:(o + 1) * 512],
                            bias_b[:, o * 512:(o + 1) * 512], Alu.add,
                        )
                    m0 = ms * MS + mi * 128
                    nc.sync.dma_start(out[m0:m0 + 128, :], out_sb[:])

    if not nc.is_finalized():
        nc.finalize()
    return nc


def _shard_inputs(x, qweight, scales, qzeros, bias):
    xt_full = np.ascontiguousarray(np.asarray(x).T)  # [K, M], replicated
    in_maps = []
    for c in range(N_CORES):
        so = slice(c * O_SHARD, (c + 1) * O_SHARD)
        sc = slice(c * C_SHARD, (c + 1) * C_SHARD)
        in_maps.append({
            "xt": xt_full,
            "qw": np.ascontiguousarray(qweight[:, sc]),
            "scales": np.ascontiguousarray(scales[:, so]),
            "qzeros": np.ascontiguousarray(qzeros[:, sc]),
            "bias": np.ascontiguousarray(bias[so]).reshape(1, -1),
        })
    return in_maps


_CACHED_NC = None


def kernel(x, qweight, scales, qzeros, bias):
    from concourse.bass_utils import run_bass_kernel_spmd

    global _CACHED_NC
    if _CACHED_NC is None:
        _CACHED_NC = build_nc()
    nc = _CACHED_NC

    in_maps = _shard_inputs(x, qweight, scales, qzeros, bias)
    res = run_bass_kernel_spmd(nc, in_maps, core_ids=list(range(N_CORES)))
    return np.concatenate([r["out"] for r in res.results], axis=1)


# revision 34
# speedup vs baseline: 1.0507x; 1.0476x over previous
"""AWQ 4-bit quantized linear layer on 8 Trainium2 NeuronCores.

Problem: out = x @ dequant(qweight, scales, qzeros) + bias
  x       [8192, 4096] fp16   (replicated to all cores, pre-transposed on host)
  qweight [4096, 1536] int32  (8x int4 nibbles packed along out_features)
  scales  [32, 12288]  fp16   (group_size=128 along in_features)
  qzeros  [32, 1536]   int32  (packed like qweight)
  bias    [12288]      fp16
  out     [8192, 12288] fp16

Sharding: tensor-parallel colwise. out_features 12288 -> 8 shards of 1536
(192 packed int32 columns). Each core computes out[:, shard] independently;
host concatenates. No collectives. x is replicated, transposed on host so
the contraction dim lands on SBUF partitions with plain (non-xbar) DMAs.

Per-core kernel:
  1. Load all packed qweight in one DMA ([128, KT, C] int32, 3 MiB).
  2. Unpack qzeros on G partitions, compute zs = z * s, stage [s | zs]
     rows to a DRAM scratch; per k-tile one 0-stride-partition DMA
     broadcasts the group's [s | zs] row to 128 partitions.
  3. Dequantize the full weight shard once into resident SBUF
     (32 tiles [128, 1536] fp16 = 96 KiB/partition):
     8x nibble isolation ops per k-tile, then w = wq * s_b - zs_b.
  4. Stream xT tiles [128, MS]; per m-tile/o-tile accumulate 32 matmuls
     in PSUM; evict with fused bias add (f32 psum + f16 bias -> f16); DMA out.
"""

import sys

for p in ("/opt/trn_rl_repo", "/opt/pypackages"):
    if p not in sys.path:
        sys.path.insert(0, p)

import numpy as np

import concourse.bacc as bacc
import concourse.bass as bass
import concourse.mybir as mybir
from concourse.tile import TileContext

f16 = mybir.dt.float16
f32 = mybir.dt.float32
i32 = mybir.dt.int32
Alu = mybir.AluOpType

N_CORES = 8
M_FULL, K_FULL, O_FULL = 8192, 4096, 12288
GROUP_SIZE = 128
PACK = 8  # int4 values per int32

O_SHARD = O_FULL // N_CORES        # 1536
C_SHARD = O_SHARD // PACK          # 192


def _perm(C):
    """Per-core column permutation: permuted position j*C + c holds the
    natural out-feature 8*c + j. Lets each nibble-unpack op write one
    contiguous C-wide block instead of a stride-8 scatter (DVE strided
    writes measured ~3x slower). scales/bias are permuted on the host;
    the output is unpermuted on the host."""
    j = np.arange(PACK).repeat(C)
    c = np.tile(np.arange(C), PACK)
    return PACK * c + j


def build_nc(M=M_FULL, K=K_FULL, O=O_SHARD, MS=512, xt_bufs=48,
             unpack_mode="staged", qw_chunk=8):
    """Build the per-core Bass program (SPMD: same program on all cores).

    Unpack is staged: (q >> 4j) & 0xF into int32 staging (bitvec ALU ops
    cannot cast on write), then one arithmetic op casts int32 -> f16.
    """
    KT = K // 128                  # k-tiles == quant groups per shard
    G = K // GROUP_SIZE
    assert KT == G, "kernel assumes group_size == 128 == k-tile"
    C = O // PACK
    OT = O // 512                  # o-tiles of 512
    NMS = M // MS                  # number of m-superchunks
    MT = MS // 128                 # m-tiles per superchunk

    # Bacc (not Bass): its compile() pipeline legalizes per-instruction
    # semaphore waits (generate_event_semaphores / move_matmul_waits_to_
    # ldweights) so walrus' per-struct sync-wait limits are respected.
    nc = bacc.Bacc("TRN2")
    xt_in = nc.dram_tensor("xt", [K, M], f16, kind="ExternalInput")
    qw = nc.dram_tensor("qw", [K, C], i32, kind="ExternalInput")
    scales = nc.dram_tensor("scales", [G, O], f16, kind="ExternalInput")
    qzeros = nc.dram_tensor("qzeros", [G, C], i32, kind="ExternalInput")
    bias = nc.dram_tensor("bias", [1, O], f16, kind="ExternalInput")
    out = nc.dram_tensor("out", [M, O], f16, kind="ExternalOutput")

    with TileContext(nc) as tc:
        with (
            tc.tile_pool(name="wres", bufs=KT) as w_pool,
            tc.tile_pool(name="xt", bufs=xt_bufs) as xt_pool,
            tc.tile_pool(name="qall", bufs=1) as qall_pool,
            tc.tile_pool(name="bc", bufs=3) as bc_pool,
            tc.tile_pool(name="meta", bufs=1) as meta_pool,
            tc.tile_pool(name="obuf", bufs=2) as o_pool,
            tc.tile_pool(name="scratch", bufs=1, space="DRAM") as dram_pool,
            tc.tile_pool(name="psum", bufs=8, space="PSUM") as psum_pool,
        ):
            assert unpack_mode == "staged"

            # dequant-phase DMAs ride the ACT HWDGE ring (nc.scalar) so they
            # never queue behind the bulk xt stream on the SP ring
            # ---- group metadata on G partitions (tiny DMAs first) ----
            # ssz row layout: [:, :O] = s, [:, O:] = zs = z * s
            # (all O-indexed tensors here use the permuted column order)
            qz_sb = meta_pool.tile([G, C], i32, tag="qz")
            nc.scalar.dma_start(qz_sb[:], qzeros[:, :])
            ssz_sb = meta_pool.tile([G, 2 * O], f16, tag="ssz")
            nc.scalar.dma_start(ssz_sb[:, :O], scales[:, :])

            qw_r = qw.rearrange("(t p) c -> p t c", p=128)
            qw_c0 = qall_pool.tile([128, qw_chunk, C], i32, tag="qwc", bufs=2)
            nc.scalar.dma_start(qw_c0[:], qw_r[:, 0:qw_chunk, :])

            zq_i = meta_pool.tile([G, O], i32, tag="zqi")
            for j in range(PACK):
                nc.vector.tensor_scalar(
                    zq_i[:, j * C:(j + 1) * C], qz_sb[:], 4 * j, 0xF,
                    Alu.logical_shift_right, Alu.bitwise_and,
                )
            # cast int32 zeros -> f16 into the zs half, then scale in place
            nc.vector.tensor_scalar(
                ssz_sb[:, O:], zq_i[:], 0, None, Alu.add)
            nc.vector.tensor_tensor(
                ssz_sb[:, O:], ssz_sb[:, O:], ssz_sb[:, :O], Alu.mult)
            ssz_dram = dram_pool.tile([G, 2 * O], f16, tag="sszd")
            nc.scalar.dma_start(ssz_dram[:, :], ssz_sb[:])

            # superchunk-0 x tiles can start now on the SP ring
            xts0 = []
            for t in range(KT):
                xt = xt_pool.tile([128, MS], f16, tag="xt", name="xt")
                nc.sync.dma_start(xt[:], xt_in[t * 128:(t + 1) * 128, 0:MS])
                xts0.append(xt)

            # ---- bias broadcast [128, O] ----
            bias_b = meta_pool.tile([128, O], f16, tag="biasb")
            nc.scalar.dma_start(bias_b[:], bias[0, :].partition_broadcast(128))

            # ---- dequantize w shard into resident SBUF tiles ----
            # packed weights arrive in chunks of qw_chunk k-tiles per DMA
            w_tiles = []
            qw_c = qw_c0
            for t in range(KT):
                if t % qw_chunk == 0 and t > 0:
                    qw_c = qall_pool.tile([128, qw_chunk, C], i32,
                                          tag="qwc", bufs=2)
                    nc.scalar.dma_start(qw_c[:], qw_r[:, t:t + qw_chunk, :])
                ssz_b = bc_pool.tile([128, 2 * O], f16, tag="sszb", bufs=2)
                nc.scalar.dma_start(
                    ssz_b[:], ssz_dram[t, :].partition_broadcast(128))
                w_t = w_pool.tile([128, O], f16, tag="w")
                wq_i = bc_pool.tile([128, O], i32, tag="wqi", bufs=2)
                for j in range(PACK):
                    nc.vector.tensor_scalar(
                        wq_i[:, j * C:(j + 1) * C], qw_c[:, t % qw_chunk, :],
                        4 * j, 0xF,
                        Alu.logical_shift_right, Alu.bitwise_and,
                    )
                # int32 -> f16 cast on ACT (own SBUF port — keeping Pool out:
                # GpSimd elementwise work locks the shared DVE port and
                # stalls the unpack), then cheap f16 2x-mode mult/sub on DVE
                wq_f = bc_pool.tile([128, O], f16, tag="wqf", bufs=2)
                nc.scalar.copy(wq_f[:], wq_i[:])
                nc.vector.tensor_tensor(
                    w_t[:], wq_f[:], ssz_b[:, :O], Alu.mult)
                nc.vector.tensor_tensor(
                    w_t[:], w_t[:], ssz_b[:, O:], Alu.subtract)
                w_tiles.append(w_t)

            # ---- main loop: stream xT, accumulate matmuls, evict ----
            for ms in range(NMS):
                if ms == 0:
                    xts = xts0
                else:
                    xts = []
                    for t in range(KT):
                        xt = xt_pool.tile([128, MS], f16, tag="xt", name="xt")
                        nc.sync.dma_start(
                            xt[:],
                            xt_in[t * 128:(t + 1) * 128,
                                  ms * MS:(ms + 1) * MS],
                        )
                        xts.append(xt)
                for mi in range(MT):
                    out_sb = o_pool.tile([128, O], f16, tag="osb")
                    for o in range(OT):
                        ps = psum_pool.tile([128, 512], f32, tag="ps")
                        for t in range(KT):
                            nc.tensor.matmul(
                                ps[:],
                                xts[t][:, mi * 128:(mi + 1) * 128],
                                w_tiles[t][:, o * 512:(o + 1) * 512],
                                start=(t == 0),
                                stop=(t == KT - 1),
                            )
                        # evict on ACT (frees the PSUM bank + DVE), then
                        # add bias in place on DVE (f16 SBUF 2x mode)
                        nc.scalar.copy(
                            out_sb[:, o * 512:(o + 1) * 512], ps[:])
                        nc.vector.tensor_tensor(
                            out_sb[:, o * 512:(o + 1) * 512],
                            out_sb[:, o * 512:(o + 1) * 512],
                            bias_b[:, o * 512:(o + 1) * 512], Alu.add,
                        )
                    m0 = ms * MS + mi * 128
                    nc.sync.dma_start(out[m0:m0 + 128, :], out_sb[:])

    if not nc.is_finalized():
        nc.finalize()
    return nc


def _shard_inputs(x, qweight, scales, qzeros, bias):
    xt_full = np.ascontiguousarray(np.asarray(x).T)  # [K, M], replicated
    perm = _perm(C_SHARD)
    in_maps = []
    for c in range(N_CORES):
        so = slice(c * O_SHARD, (c + 1) * O_SHARD)
        sc = slice(c * C_SHARD, (c + 1) * C_SHARD)
        in_maps.append({
            "xt": xt_full,
            "qw": np.ascontiguousarray(qweight[:, sc]),
            "scales": np.ascontiguousarray(scales[:, so][:, perm]),
            "qzeros": np.ascontiguousarray(qzeros[:, sc]),
            "bias": np.ascontiguousarray(bias[so][perm]).reshape(1, -1),
        })
    return in_maps


_CACHED_NC = None


def kernel(x, qweight, scales, qzeros, bias):
    from concourse.bass_utils import run_bass_kernel_spmd

    global _CACHED_NC
    if _CACHED_NC is None:
        _CACHED_NC = build_nc()
    nc = _CACHED_NC

    in_maps = _shard_inputs(x, qweight, scales, qzeros, bias)
    res = run_bass_kernel_spmd(nc, in_maps, core_ids=list(range(N_CORES)))
    # undo the per-core column permutation while gathering
    perm = _perm(C_SHARD)
    out = np.empty((M_FULL, O_FULL), dtype=np.float16)
    for c in range(N_CORES):
        out[:, c * O_SHARD + perm] = res.results[c]["out"]
    return out
